# revision 81
# baseline (speedup 1.0000x reference)
"""Trainium2 Bass kernel for ConvSelfAttention (B=4, C=128, W=H=64).

Reference computation (per batch b, with N = W*H = 4096):
    q = wq @ x + bq ; k = wk @ x + bk ; v = wv @ x + bv        # [C, N]
    S[n, m] = (q[:, n] . k[:, m]) / sqrt(C)
    A = softmax(S, axis=m)                                     # [N, N]
    out[c, n] = sum_m v[c, m] A[n, m]
    y = wo @ out + bo
    result = gamma * y + x

Sharding: 8 cores = 4 batches x 2 halves of the attention-row dim n.
Each core holds full x[b] (for k, v) and computes out[:, n_slice].
No collectives needed; host gathers the slices.

Kernel-internal layout trick: scores are computed TRANSPOSED,
S_T[m, n] = k_tile^T @ q, so that m (the softmax-reduction dim) lands on
partitions.  exp(S_T) tiles then feed the P@V matmul directly as the
moving operand (contraction over m = partitions) producing out[c, n]
with channels on partitions — which is exactly what the final projection
needs.  Softmax denominators come from an elementwise accumulation of
the exp tiles (VectorE) followed by a ones-vector matmul partition
reduction; normalization is a broadcast multiply.

Host-side folding (all cheap [C,C] ops):
  - 1/sqrt(C) scale folded into wq, bq.
  - v bias: since softmax rows sum to 1, v's bias contributes exactly
    bv[c] to the attention output, so it is folded into the output
    projection bias:  bo_eff = wo @ bv + bo.
  - gamma folded into wo and bo_eff.
"""

import math
import os
import sys

import numpy as np

if "/opt/trn_rl_repo" not in sys.path:
    sys.path.insert(0, "/opt/trn_rl_repo")

B, C, W, H = 4, 128, 64, 64
N = W * H            # 4096
HALF = N // 2        # 2048 n-columns per core
CHUNK = 512          # n-columns per PSUM bank (fp32)
BIG = 1024           # n-columns per exp/score group (2 PSUM banks)
NBIG = HALF // BIG   # 2 big chunks per core
MT = N // 128        # 32 m-tiles of the key/value positions
LOOKAHEAD = 1        # score-group software lookahead (ps_s bufs = LOOKAHEAD+1)
ROUND_BITS = 19      # host-side round-to-nearest mantissa bits for f32r inputs

# "f32r" = relaxed-precision fp32 matmul (4x faster than strict fp32 on the
# PE when the moving dim is >= 256).  Walrus requires f32r matmul operands
# to be *produced* as f32r (rounded) by a compute op, so the big-matmul
# operand tiles (k, q, vT, expS) are allocated as float32r and written by
# DVE/ACT ops; the small projection matmuls stay strict fp32.
MM_DTYPE = "f32r"

_BUILT = {}


def _build(mm_dtype=MM_DTYPE):
    """Build + compile the single-core Bass/Tile program (shared SPMD)."""
    if mm_dtype in _BUILT:
        return _BUILT[mm_dtype]

    import concourse.bass as bass
    from concourse import bacc, mybir
    from concourse.tile import TileContext

    f32 = mybir.dt.float32
    mmdt = mybir.dt.float32r if mm_dtype == "f32r" else mybir.dt.float32

    nc = bacc.Bacc("TRN2", target_bir_lowering=False)

    # Inputs arrive as plain fp32; operands of f32r matmuls are rounded
    # on-device by cheap DVE copies (hardware f32r matmuls lose precision
    # on operands that were not engine-rounded: measured 1e-4 vs 1e-5).
    # All weights/biases travel in ONE packed tensor: each separate DMA
    # pays ~2us of HBM completion latency.
    x_d = nc.dram_tensor("x", [C, N], f32, kind="ExternalInput")
    xq_d = nc.dram_tensor("xq", [C, HALF], f32, kind="ExternalInput")
    wp_d = nc.dram_tensor("wpack", [C, 4 * C + 3], f32, kind="ExternalInput")
    y_d = nc.dram_tensor("y", [C, HALF], f32, kind="ExternalOutput")

    f16 = mybir.dt.float16

    with TileContext(nc) as tc:
        with (
            tc.tile_pool(name="consts", bufs=1) as consts,
            tc.tile_pool(name="bigs", bufs=1) as bigs,
            tc.tile_pool(name="exps", bufs=8) as exps,
            tc.tile_pool(name="tre16", bufs=5) as tre16,
            tc.tile_pool(name="tre32", bufs=2) as tre32,
            tc.tile_pool(name="smalls", bufs=2) as smalls,
            tc.tile_pool(name="outs", bufs=3) as outs,
            tc.tile_pool(name="xsf", bufs=5) as xsf,
            tc.tile_pool(name="xs", bufs=3) as xs,
            tc.tile_pool(name="ps_s", bufs=LOOKAHEAD + 1, space="PSUM") as pp_s,
            tc.tile_pool(name="ps_o", bufs=3, space="PSUM") as pp_o,
            tc.tile_pool(name="ps_misc", bufs=1, space="PSUM") as pp_misc,
        ):
            # ---- constants & inputs -------------------------------------
            wp_sb = consts.tile([C, 4 * C + 3], f32, tag="wp")
            wq_sb = consts.tile([C, C], mmdt, tag="wq")
            wk_sb = consts.tile([C, C], mmdt, tag="wk")
            wv_h = consts.tile([C, C], f16, tag="wv")
            wo_r = consts.tile([C, C], mmdt, tag="wor")
            ones_col = consts.tile([C, 1], f32, tag="onc")
            ones_row = consts.tile([1, C], f32, tag="onr")
            ones_col_r = consts.tile([C, 1], mmdt, tag="oncr")
            ones_row_r = consts.tile([1, C], mmdt, tag="onrr")

            xq_sb = bigs.tile([C, HALF], f32, tag="xq")
            xq_r = bigs.tile([C, HALF], mmdt, tag="xqr")
            k_sb = bigs.tile([C, N], mmdt, tag="k")
            q_sb = bigs.tile([C, HALF], mmdt, tag="q")
            vT_sb = bigs.tile([128, MT, C], f16, tag="vT")
            outN_sb = bigs.tile([C, HALF], mmdt, tag="outN")

            # sync HWDGE ring: weights, xq.  ACT HWDGE ring: x pieces —
            # the two rings run in parallel.
            # wq|wk (first 2C columns) land first — they gate the first
            # score matmul; the rest of the pack follows the first xq chunk.
            nc.sync.dma_start(out=wp_sb[:, :2 * C], in_=wp_d[:, :2 * C])
            nc.sync.dma_start(out=xq_sb[:, :CHUNK], in_=xq_d[:, :CHUNK])
            nc.sync.dma_start(out=wp_sb[:, 2 * C:], in_=wp_d[:, 2 * C:])
            nc.sync.dma_start(out=xq_sb[:, CHUNK:], in_=xq_d[:, CHUNK:])
            wo_sb = wp_sb[:, 3 * C:4 * C]
            bq_sb = wp_sb[:, 4 * C:4 * C + 1]
            bk_sb = wp_sb[:, 4 * C + 1:4 * C + 2]
            bo_sb = wp_sb[:, 4 * C + 2:4 * C + 3]
            # DVE FIFO order matters here: only the copies on the
            # first-exp critical path (wq, wk, first xq chunk) go first.
            nc.vector.tensor_copy(wq_sb, wp_sb[:, 0:C])
            nc.vector.tensor_copy(wk_sb, wp_sb[:, C:2 * C])
            nc.vector.tensor_copy(xq_r[:, :CHUNK], xq_sb[:, :CHUNK])

            # ---- projections, pipelined against the x DMA ---------------
            # x arrives in BIG-wide pieces on the ACT DMA ring; k and vT
            # consume each piece as soon as it lands (x itself is not
            # needed afterwards).  Piece 0 + q are emitted up front; pieces
            # 1..3 are interleaved into the job stream right before the
            # first score matmul that needs their k tiles, so the attention
            # stream starts while later pieces are still in flight.
            # pieces: a narrow one first (fast path to the first scores),
            # then wide ones.  All DMAs are issued up front (piece 0 on the
            # ACT ring, in parallel with wpack/xq on the sync ring); the
            # projection matmuls are drained one small bundle per job so the
            # exp stream never starves.
            pieces = [(0, 512), (512, 512), (1024, 1024), (2048, 1024), (3072, 1024)]
            piece_dma = []
            for pi, (n0, w) in enumerate(pieces):
                xp_f = xsf.tile([C, w], f32, tag="xpf", name=f"xpf{pi}")
                eng = nc.scalar if pi == 0 else nc.sync
                eng.dma_start(out=xp_f, in_=x_d[:, bass.ds(n0, w)])
                piece_dma.append(xp_f)

            def piece_bundles(pi, n0, w):
                xp_f = piece_dma[pi]
                xp = xs.tile([C, w], mmdt, tag="xp", name=f"xp{pi}")
                xp_h = xs.tile([C, w], f16, tag="xph", name=f"xph{pi}")

                def copies():
                    nc.vector.tensor_copy(xp, xp_f)
                    nc.vector.tensor_copy(xp_h, xp_f)

                def k_half(h):
                    ps_k = pp_o.tile([128, CHUNK], f32, tag="o",
                                     name=f"psk{pi}_{h}")
                    nc.tensor.matmul(
                        ps_k, wk_sb, xp[:, bass.ts(h, CHUNK)],
                        start=True, stop=True,
                    )
                    nc.vector.tensor_scalar_add(
                        k_sb[:, bass.ds(n0 + h * CHUNK, CHUNK)], ps_k, bk_sb
                    )

                def v_quarter(g):
                    ps_v = pp_o.tile([128, CHUNK], f32, tag="o",
                                     name=f"psv{pi}_{g}")
                    for tt in range(4):
                        nc.tensor.matmul(
                            ps_v[:, bass.ts(tt, 128)],
                            xp_h[:, bass.ds(g * CHUNK + tt * 128, 128)],
                            wv_h, start=True, stop=True,
                        )
                    nc.vector.tensor_copy(
                        vT_sb[:, bass.ds(n0 // 128 + g * 4, 4), :],
                        ps_v.rearrange("p (t c) -> p t c", c=C),
                    )

                out = [copies]
                for h in range(w // CHUNK):
                    out.append(lambda h=h: k_half(h))
                    out.append(lambda g=h: v_quarter(g))
                return out

            pb = {pi: piece_bundles(pi, *pieces[pi])
                  for pi in range(len(pieces))}
            # piece 0 runs up front: copies + k first (q + the first scores
            # depend on them), vT afterwards
            p0 = pb[0]
            for fn in p0[:2]:
                fn()
            def q_group(j):
                ps_q = pp_o.tile([128, CHUNK], f32, tag="o", name=f"psq{j}")
                nc.tensor.matmul(
                    ps_q, wq_sb, xq_r[:, bass.ts(j, CHUNK)],
                    start=True, stop=True,
                )
                nc.vector.tensor_scalar_add(
                    q_sb[:, bass.ts(j, CHUNK)], ps_q, bq_sb
                )

            q_group(0)
            nc.vector.tensor_copy(wv_h, wp_sb[:, 2 * C:3 * C])
            p0[2]()  # piece 0's vT, needed by the first PV
            # everything else is off the first-exp critical path; the
            # interleave below meets every k/vT/q deadline at one bundle
            # per job while spreading the PE load evenly
            bundles = (
                [lambda: q_group(1)] + pb[1] + pb[2]
                + [lambda: q_group(2)] + pb[3]
                + [lambda: q_group(3)] + pb[4]
            )
            nc.vector.tensor_copy(xq_r[:, CHUNK:], xq_sb[:, CHUNK:])
            nc.vector.memset(ones_col, 1.0)
            nc.vector.memset(ones_row, 1.0)
            nc.vector.tensor_copy(ones_col_r, ones_col)
            nc.vector.tensor_copy(ones_row_r, ones_row)
            nc.vector.tensor_copy(wo_r, wp_sb[:, 3 * C:4 * C])

            # ---- attention main loop ------------------------------------
            # n-groups of decreasing width (the last, narrow chunks keep the
            # exposed epilogue short); softmax denominators come from a
            # binary-tree reduction of the exp tiles (fp16 at the two lowest
            # levels, fp32 above) + a ones-vector partition reduce.
            chunks = [(0, CHUNK), (CHUNK, BIG), (CHUNK + BIG, CHUNK)]
            jobs = [(ci, t) for ci in range(len(chunks)) for t in range(MT)]
            pending = {}

            def emit_scores(ci, t):
                n0, w = chunks[ci]
                ps_s = pp_s.tile([128, w], f32, tag="s", name=f"s{ci}_{t}")
                # S_T[m_tile, n_group] = k_tile^T @ q_group (512-col halves)
                for h in range(w // CHUNK):
                    nc.tensor.matmul(
                        ps_s[:, bass.ts(h, CHUNK)],
                        k_sb[:, bass.ts(t, 128)],
                        q_sb[:, bass.ds(n0 + h * CHUNK, CHUNK)],
                        start=True, stop=True,
                    )
                return ps_s

            for i in range(min(LOOKAHEAD, len(jobs))):
                pending[jobs[i]] = emit_scores(*jobs[i])

            psum_o = []
            levels = []
            for i, (ci, t) in enumerate(jobs):
                if i >= 1 and bundles:
                    bundles.pop(0)()
                n0, w = chunks[ci]
                nh = w // CHUNK
                if t == 0:
                    psum_o = [
                        pp_o.tile([128, CHUNK], f32, tag="o", name=f"o{ci}_{h}")
                        for h in range(nh)
                    ]
                    levels = [[] for _ in range(8)]
                ps_s = pending.pop((ci, t))
                expS = exps.tile([128, w], f16, tag="e", name=f"e{ci}_{t}")
                nc.scalar.activation(
                    expS, ps_s, mybir.ActivationFunctionType.Exp
                )
                if i + LOOKAHEAD < len(jobs):
                    nj = jobs[i + LOOKAHEAD]
                    pending[nj] = emit_scores(*nj)
                # out0[c, n] += v_tile @ expS_tile   (contract over m)
                for h in range(nh):
                    nc.tensor.matmul(
                        psum_o[h], vT_sb[:, t, :],
                        expS[:, bass.ts(h, CHUNK)],
                        start=(t == 0), stop=(t == MT - 1),
                    )
                # tree accumulation of exp tiles for the denominators
                levels[0].append(expS)
                lvl = 0
                while len(levels[lvl]) >= 2:
                    a = levels[lvl].pop(0)
                    b = levels[lvl].pop(0)
                    if lvl < 2:
                        s = tre16.tile([128, w], f16, tag=f"l{lvl}",
                                       name=f"a{ci}_{i}_{lvl}")
                    elif lvl < 4:
                        s = tre32.tile([128, w], f32, tag=f"l{lvl}",
                                       name=f"a{ci}_{i}_{lvl}")
                    else:
                        # top level in f32r so the ones-reduce matmul is cheap
                        s = tre32.tile([128, w], mmdt, tag=f"l{lvl}",
                                       name=f"a{ci}_{i}_{lvl}")
                    nc.vector.tensor_add(s, a, b)
                    levels[lvl + 1].append(s)
                    lvl += 1

                if t == MT - 1:
                    u = levels[5][0]  # [128, w] f32r total over all m-tiles
                    for h in range(nh):
                        cn = n0 + h * CHUNK
                        ps_sum = pp_misc.tile([128, CHUNK], f32, tag="misc")
                        nc.tensor.matmul(
                            ps_sum[:1, :], ones_col_r, u[:, bass.ts(h, CHUNK)],
                            start=True, stop=True,
                        )
                        ssum = smalls.tile([1, CHUNK], mmdt, tag="ssum")
                        nc.vector.tensor_copy(ssum, ps_sum[:1, :])
                        ps_b = pp_misc.tile([128, CHUNK], f32, tag="misc")
                        nc.tensor.matmul(
                            ps_b, ones_row_r, ssum, start=True, stop=True
                        )
                        rb = smalls.tile([128, CHUNK], f32, tag="rb")
                        nc.vector.reciprocal_approx_fast(rb, ps_b)
                        nc.vector.tensor_mul(
                            outN_sb[:, bass.ds(cn, CHUNK)], psum_o[h], rb
                        )
                        # output projection + fused bias/residual add
                        ps_y = pp_misc.tile([128, CHUNK], f32, tag="misc")
                        nc.tensor.matmul(
                            ps_y, wo_r, outN_sb[:, bass.ds(cn, CHUNK)],
                            start=True, stop=True,
                        )
                        t2 = outs.tile([128, CHUNK], f32, tag="t2")
                        nc.vector.scalar_tensor_tensor(
                            t2, ps_y, bo_sb, xq_sb[:, bass.ds(cn, CHUNK)],
                            op0=mybir.AluOpType.add, op1=mybir.AluOpType.add,
                        )
                        nc.sync.dma_start(
                            out=y_d[:, bass.ds(cn, CHUNK)], in_=t2
                        )

    nc.compile()
    _BUILT[mm_dtype] = nc
    return nc


def _round_mant(a, bits=ROUND_BITS):
    """Round fp32 mantissa to `bits` explicit bits (round-to-nearest-even).

    The PE's relaxed-fp32 (f32r) path drops low mantissa bits of operands
    that were not pre-rounded; rounding on the host (free) instead of
    letting the hardware truncate removes the truncation bias.
    """
    drop = 23 - bits
    u = a.astype(np.float32).view(np.uint32)
    round_bit = np.uint32(1 << (drop - 1))
    lsb = (u >> np.uint32(drop)) & np.uint32(1)
    u = u + (round_bit - np.uint32(1)) + lsb
    u &= np.uint32(~((1 << drop) - 1) & 0xFFFFFFFF)
    return u.view(np.float32)


def _make_in_maps(inputs):
    x = np.asarray(inputs["x"], np.float32)
    wq = np.asarray(inputs["wq"], np.float32)
    bq = np.asarray(inputs["bq"], np.float32)
    wk = np.asarray(inputs["wk"], np.float32)
    bk = np.asarray(inputs["bk"], np.float32)
    wv = np.asarray(inputs["wv"], np.float32)
    bv = np.asarray(inputs["bv"], np.float32)
    wo = np.asarray(inputs["wo"], np.float32)
    bo = np.asarray(inputs["bo"], np.float32)
    gamma = float(np.asarray(inputs["gamma"], np.float32)[0])

    s = 1.0 / math.sqrt(C)
    wpack = np.ascontiguousarray(np.hstack([
        (wq * s).T, wk.T, wv.T, (wo * gamma).T,
        (bq * s).reshape(C, 1), bk.reshape(C, 1),
        (gamma * (wo @ bv + bo)).reshape(C, 1),
    ]).astype(np.float32))

    xf = np.ascontiguousarray(x.reshape(B, C, N))
    in_maps = []
    for core in range(8):
        b, half = core // 2, core % 2
        in_maps.append({
            "x": xf[b],
            "xq": np.ascontiguousarray(xf[b][:, half * HALF:(half + 1) * HALF]),
            "wpack": wpack,
        })
    return in_maps


def _gather(results):
    out = np.empty((B, C, N), np.float32)
    for core in range(8):
        b, half = core // 2, core % 2
        out[b][:, half * HALF:(half + 1) * HALF] = results[core]["y"]
    return out.reshape(B, C, W, H)


def run(inputs, trace=False):
    """Run on the 8 NeuronCores; returns (output, exec_time_ns_or_None)."""
    from concourse.bass_utils import run_bass_kernel_spmd

    nc = _build()
    in_maps = _make_in_maps(inputs)
    res = run_bass_kernel_spmd(nc, in_maps, core_ids=list(range(8)), trace=trace)
    return _gather(res.results), res.exec_time_ns


def kernel(**inputs):
    out, _ = run(inputs)
    return out


# revision 82
# speedup vs baseline: 1.0223x; 1.0223x over previous
"""Trainium2 Bass kernel for ConvSelfAttention (B=4, C=128, W=H=64).

Reference computation (per batch b, with N = W*H = 4096):
    q = wq @ x + bq ; k = wk @ x + bk ; v = wv @ x + bv        # [C, N]
    S[n, m] = (q[:, n] . k[:, m]) / sqrt(C)
    A = softmax(S, axis=m)                                     # [N, N]
    out[c, n] = sum_m v[c, m] A[n, m]
    y = wo @ out + bo
    result = gamma * y + x

Sharding: 8 cores = 4 batches x 2 halves of the attention-row dim n.
Each core holds full x[b] (for k, v) and computes out[:, n_slice].
No collectives needed; host gathers the slices.

Kernel-internal layout trick: scores are computed TRANSPOSED,
S_T[m, n] = k_tile^T @ q, so that m (the softmax-reduction dim) lands on
partitions.  exp(S_T) tiles then feed the P@V matmul directly as the
moving operand (contraction over m = partitions) producing out[c, n]
with channels on partitions — which is exactly what the final projection
needs.  Softmax denominators come from an elementwise accumulation of
the exp tiles (VectorE) followed by a ones-vector matmul partition
reduction; normalization is a broadcast multiply.

Host-side folding (all cheap [C,C] ops):
  - 1/sqrt(C) scale folded into wq, bq.
  - v bias: since softmax rows sum to 1, v's bias contributes exactly
    bv[c] to the attention output, so it is folded into the output
    projection bias:  bo_eff = wo @ bv + bo.
  - gamma folded into wo and bo_eff.
"""

import math
import os
import sys

import numpy as np

if "/opt/trn_rl_repo" not in sys.path:
    sys.path.insert(0, "/opt/trn_rl_repo")

B, C, W, H = 4, 128, 64, 64
N = W * H            # 4096
HALF = N // 2        # 2048 n-columns per core
CHUNK = 512          # n-columns per PSUM bank (fp32)
BIG = 1024           # n-columns per exp/score group (2 PSUM banks)
NBIG = HALF // BIG   # 2 big chunks per core
MT = N // 128        # 32 m-tiles of the key/value positions
LOOKAHEAD = 1        # score-group software lookahead (ps_s bufs = LOOKAHEAD+1)
ROUND_BITS = 19      # host-side round-to-nearest mantissa bits for f32r inputs

# "f32r" = relaxed-precision fp32 matmul (4x faster than strict fp32 on the
# PE when the moving dim is >= 256).  Walrus requires f32r matmul operands
# to be *produced* as f32r (rounded) by a compute op, so the big-matmul
# operand tiles (k, q, vT, expS) are allocated as float32r and written by
# DVE/ACT ops; the small projection matmuls stay strict fp32.
MM_DTYPE = "f32r"

_BUILT = {}


def _build(mm_dtype=MM_DTYPE):
    """Build + compile the single-core Bass/Tile program (shared SPMD)."""
    if mm_dtype in _BUILT:
        return _BUILT[mm_dtype]

    import concourse.bass as bass
    from concourse import bacc, mybir
    from concourse.tile import TileContext

    f32 = mybir.dt.float32
    mmdt = mybir.dt.float32r if mm_dtype == "f32r" else mybir.dt.float32

    nc = bacc.Bacc("TRN2", target_bir_lowering=False)

    # Inputs arrive as plain fp32; operands of f32r matmuls are rounded
    # on-device by cheap DVE copies (hardware f32r matmuls lose precision
    # on operands that were not engine-rounded: measured 1e-4 vs 1e-5).
    # All weights/biases travel in ONE packed tensor: each separate DMA
    # pays ~2us of HBM completion latency.
    x_d = nc.dram_tensor("x", [C, N], f32, kind="ExternalInput")
    xq_d = nc.dram_tensor("xq", [C, HALF], f32, kind="ExternalInput")
    wp_d = nc.dram_tensor("wpack", [C, 4 * C + 3], f32, kind="ExternalInput")
    y_d = nc.dram_tensor("y", [C, HALF], f32, kind="ExternalOutput")

    f16 = mybir.dt.float16

    with TileContext(nc) as tc:
        with (
            tc.tile_pool(name="consts", bufs=1) as consts,
            tc.tile_pool(name="bigs", bufs=1) as bigs,
            tc.tile_pool(name="exps", bufs=8) as exps,
            tc.tile_pool(name="tre16", bufs=5) as tre16,
            tc.tile_pool(name="tre32", bufs=2) as tre32,
            tc.tile_pool(name="smalls", bufs=2) as smalls,
            tc.tile_pool(name="outs", bufs=3) as outs,
            tc.tile_pool(name="xsf", bufs=5) as xsf,
            tc.tile_pool(name="xs", bufs=3) as xs,
            tc.tile_pool(name="ps_s", bufs=LOOKAHEAD + 1, space="PSUM") as pp_s,
            tc.tile_pool(name="ps_o", bufs=3, space="PSUM") as pp_o,
            tc.tile_pool(name="ps_misc", bufs=1, space="PSUM") as pp_misc,
        ):
            # ---- constants & inputs -------------------------------------
            wp_sb = consts.tile([C, 4 * C + 3], f32, tag="wp")
            wq_sb = consts.tile([C, C], mmdt, tag="wq")
            wk_sb = consts.tile([C, C], mmdt, tag="wk")
            wv_h = consts.tile([C, C], f16, tag="wv")
            wo_r = consts.tile([C, C], mmdt, tag="wor")
            ones_col = consts.tile([C, 1], f32, tag="onc")
            ones_row = consts.tile([1, C], f32, tag="onr")
            ones_col_r = consts.tile([C, 1], mmdt, tag="oncr")
            ones_row_r = consts.tile([1, C], mmdt, tag="onrr")

            xq_sb = bigs.tile([C, HALF], f32, tag="xq")
            xq_r = bigs.tile([C, HALF], mmdt, tag="xqr")
            k_sb = bigs.tile([C, N], mmdt, tag="k")
            q_sb = bigs.tile([C, HALF], mmdt, tag="q")
            vT_sb = bigs.tile([128, MT, C], f16, tag="vT")
            outN_sb = bigs.tile([C, HALF], mmdt, tag="outN")

            # sync HWDGE ring: weights, xq.  ACT HWDGE ring: x pieces —
            # the two rings run in parallel.
            nc.sync.dma_start(out=wp_sb, in_=wp_d[:, :])
            nc.sync.dma_start(out=xq_sb[:, :CHUNK], in_=xq_d[:, :CHUNK])
            nc.sync.dma_start(out=xq_sb[:, CHUNK:], in_=xq_d[:, CHUNK:])
            wo_sb = wp_sb[:, 3 * C:4 * C]
            bq_sb = wp_sb[:, 4 * C:4 * C + 1]
            bk_sb = wp_sb[:, 4 * C + 1:4 * C + 2]
            bo_sb = wp_sb[:, 4 * C + 2:4 * C + 3]
            # DVE FIFO order matters here: only the copies on the
            # first-exp critical path (wq, wk, first xq chunk) go first.
            nc.vector.tensor_copy(wq_sb, wp_sb[:, 0:C])
            nc.vector.tensor_copy(wk_sb, wp_sb[:, C:2 * C])
            nc.vector.tensor_copy(xq_r[:, :CHUNK], xq_sb[:, :CHUNK])

            # ---- projections, pipelined against the x DMA ---------------
            # x arrives in BIG-wide pieces on the ACT DMA ring; k and vT
            # consume each piece as soon as it lands (x itself is not
            # needed afterwards).  Piece 0 + q are emitted up front; pieces
            # 1..3 are interleaved into the job stream right before the
            # first score matmul that needs their k tiles, so the attention
            # stream starts while later pieces are still in flight.
            # pieces: a narrow one first (fast path to the first scores),
            # then wide ones.  All DMAs are issued up front (piece 0 on the
            # ACT ring, in parallel with wpack/xq on the sync ring); the
            # projection matmuls are drained one small bundle per job so the
            # exp stream never starves.
            pieces = [(0, 512), (512, 512), (1024, 1024), (2048, 1024), (3072, 1024)]
            piece_dma = []
            for pi, (n0, w) in enumerate(pieces):
                xp_f = xsf.tile([C, w], f32, tag="xpf", name=f"xpf{pi}")
                eng = nc.scalar if pi == 0 else nc.sync
                eng.dma_start(out=xp_f, in_=x_d[:, bass.ds(n0, w)])
                piece_dma.append(xp_f)

            def piece_bundles(pi, n0, w):
                xp_f = piece_dma[pi]
                xp = xs.tile([C, w], mmdt, tag="xp", name=f"xp{pi}")
                xp_h = xs.tile([C, w], f16, tag="xph", name=f"xph{pi}")

                def copies():
                    nc.vector.tensor_copy(xp, xp_f)
                    nc.vector.tensor_copy(xp_h, xp_f)

                def k_half(h):
                    ps_k = pp_o.tile([128, CHUNK], f32, tag="o",
                                     name=f"psk{pi}_{h}")
                    nc.tensor.matmul(
                        ps_k, wk_sb, xp[:, bass.ts(h, CHUNK)],
                        start=True, stop=True,
                    )
                    nc.vector.tensor_scalar_add(
                        k_sb[:, bass.ds(n0 + h * CHUNK, CHUNK)], ps_k, bk_sb
                    )

                def v_quarter(g):
                    ps_v = pp_o.tile([128, CHUNK], f32, tag="o",
                                     name=f"psv{pi}_{g}")
                    for tt in range(4):
                        nc.tensor.matmul(
                            ps_v[:, bass.ts(tt, 128)],
                            xp_h[:, bass.ds(g * CHUNK + tt * 128, 128)],
                            wv_h, start=True, stop=True,
                        )
                    nc.vector.tensor_copy(
                        vT_sb[:, bass.ds(n0 // 128 + g * 4, 4), :],
                        ps_v.rearrange("p (t c) -> p t c", c=C),
                    )

                out = [copies]
                for h in range(w // CHUNK):
                    out.append(lambda h=h: k_half(h))
                    out.append(lambda g=h: v_quarter(g))
                return out

            pb = {pi: piece_bundles(pi, *pieces[pi])
                  for pi in range(len(pieces))}
            # piece 0 runs up front: copies + k first (q + the first scores
            # depend on them), vT afterwards
            p0 = pb[0]
            for fn in p0[:2]:
                fn()
            def q_group(j):
                ps_q = pp_o.tile([128, CHUNK], f32, tag="o", name=f"psq{j}")
                nc.tensor.matmul(
                    ps_q, wq_sb, xq_r[:, bass.ts(j, CHUNK)],
                    start=True, stop=True,
                )
                nc.vector.tensor_scalar_add(
                    q_sb[:, bass.ts(j, CHUNK)], ps_q, bq_sb
                )

            q_group(0)
            nc.vector.tensor_copy(wv_h, wp_sb[:, 2 * C:3 * C])
            p0[2]()  # piece 0's vT, needed by the first PV
            # everything else is off the first-exp critical path; the
            # interleave below meets every k/vT/q deadline at one bundle
            # per job while spreading the PE load evenly
            bundles = (
                [lambda: q_group(1)] + pb[1] + pb[2]
                + [lambda: q_group(2)] + pb[3]
                + [lambda: q_group(3)] + pb[4]
            )
            nc.vector.tensor_copy(xq_r[:, CHUNK:], xq_sb[:, CHUNK:])
            nc.vector.memset(ones_col, 1.0)
            nc.vector.memset(ones_row, 1.0)
            nc.vector.tensor_copy(ones_col_r, ones_col)
            nc.vector.tensor_copy(ones_row_r, ones_row)
            nc.vector.tensor_copy(wo_r, wp_sb[:, 3 * C:4 * C])

            # ---- attention main loop ------------------------------------
            # n-groups of decreasing width (the last, narrow chunks keep the
            # exposed epilogue short); softmax denominators come from a
            # binary-tree reduction of the exp tiles (fp16 at the two lowest
            # levels, fp32 above) + a ones-vector partition reduce.
            chunks = [(0, CHUNK), (CHUNK, BIG), (CHUNK + BIG, CHUNK)]
            jobs = [(ci, t) for ci in range(len(chunks)) for t in range(MT)]
            pending = {}

            def emit_scores(ci, t):
                n0, w = chunks[ci]
                ps_s = pp_s.tile([128, w], f32, tag="s", name=f"s{ci}_{t}")
                # S_T[m_tile, n_group] = k_tile^T @ q_group (512-col halves)
                for h in range(w // CHUNK):
                    nc.tensor.matmul(
                        ps_s[:, bass.ts(h, CHUNK)],
                        k_sb[:, bass.ts(t, 128)],
                        q_sb[:, bass.ds(n0 + h * CHUNK, CHUNK)],
                        start=True, stop=True,
                    )
                return ps_s

            for i in range(min(LOOKAHEAD, len(jobs))):
                pending[jobs[i]] = emit_scores(*jobs[i])

            psum_o = []
            levels = []
            for i, (ci, t) in enumerate(jobs):
                if i >= 1 and bundles:
                    bundles.pop(0)()
                n0, w = chunks[ci]
                nh = w // CHUNK
                if t == 0:
                    psum_o = [
                        pp_o.tile([128, CHUNK], f32, tag="o", name=f"o{ci}_{h}")
                        for h in range(nh)
                    ]
                    levels = [[] for _ in range(8)]
                ps_s = pending.pop((ci, t))
                expS = exps.tile([128, w], f16, tag="e", name=f"e{ci}_{t}")
                nc.scalar.activation(
                    expS, ps_s, mybir.ActivationFunctionType.Exp
                )
                if i + LOOKAHEAD < len(jobs):
                    nj = jobs[i + LOOKAHEAD]
                    pending[nj] = emit_scores(*nj)
                # out0[c, n] += v_tile @ expS_tile   (contract over m)
                for h in range(nh):
                    nc.tensor.matmul(
                        psum_o[h], vT_sb[:, t, :],
                        expS[:, bass.ts(h, CHUNK)],
                        start=(t == 0), stop=(t == MT - 1),
                    )
                # tree accumulation of exp tiles for the denominators
                levels[0].append(expS)
                lvl = 0
                while len(levels[lvl]) >= 2:
                    a = levels[lvl].pop(0)
                    b = levels[lvl].pop(0)
                    if lvl < 2:
                        s = tre16.tile([128, w], f16, tag=f"l{lvl}",
                                       name=f"a{ci}_{i}_{lvl}")
                    elif lvl < 4:
                        s = tre32.tile([128, w], f32, tag=f"l{lvl}",
                                       name=f"a{ci}_{i}_{lvl}")
                    else:
                        # top level in f32r so the ones-reduce matmul is cheap
                        s = tre32.tile([128, w], mmdt, tag=f"l{lvl}",
                                       name=f"a{ci}_{i}_{lvl}")
                    nc.vector.tensor_add(s, a, b)
                    levels[lvl + 1].append(s)
                    lvl += 1

                if t == MT - 1:
                    u = levels[5][0]  # [128, w] f32r total over all m-tiles
                    for h in range(nh):
                        cn = n0 + h * CHUNK
                        ps_sum = pp_misc.tile([128, CHUNK], f32, tag="misc")
                        nc.tensor.matmul(
                            ps_sum[:1, :], ones_col_r, u[:, bass.ts(h, CHUNK)],
                            start=True, stop=True,
                        )
                        ssum = smalls.tile([1, CHUNK], mmdt, tag="ssum")
                        nc.vector.tensor_copy(ssum, ps_sum[:1, :])
                        ps_b = pp_misc.tile([128, CHUNK], f32, tag="misc")
                        nc.tensor.matmul(
                            ps_b, ones_row_r, ssum, start=True, stop=True
                        )
                        rb = smalls.tile([128, CHUNK], f32, tag="rb")
                        nc.vector.reciprocal_approx_fast(rb, ps_b)
                        nc.vector.tensor_mul(
                            outN_sb[:, bass.ds(cn, CHUNK)], psum_o[h], rb
                        )
                        # output projection + fused bias/residual add
                        ps_y = pp_misc.tile([128, CHUNK], f32, tag="misc")
                        nc.tensor.matmul(
                            ps_y, wo_r, outN_sb[:, bass.ds(cn, CHUNK)],
                            start=True, stop=True,
                        )
                        t2 = outs.tile([128, CHUNK], f32, tag="t2")
                        nc.vector.scalar_tensor_tensor(
                            t2, ps_y, bo_sb, xq_sb[:, bass.ds(cn, CHUNK)],
                            op0=mybir.AluOpType.add, op1=mybir.AluOpType.add,
                        )
                        nc.sync.dma_start(
                            out=y_d[:, bass.ds(cn, CHUNK)], in_=t2
                        )

    nc.compile()
    _BUILT[mm_dtype] = nc
    return nc


def _round_mant(a, bits=ROUND_BITS):
    """Round fp32 mantissa to `bits` explicit bits (round-to-nearest-even).

    The PE's relaxed-fp32 (f32r) path drops low mantissa bits of operands
    that were not pre-rounded; rounding on the host (free) instead of
    letting the hardware truncate removes the truncation bias.
    """
    drop = 23 - bits
    u = a.astype(np.float32).view(np.uint32)
    round_bit = np.uint32(1 << (drop - 1))
    lsb = (u >> np.uint32(drop)) & np.uint32(1)
    u = u + (round_bit - np.uint32(1)) + lsb
    u &= np.uint32(~((1 << drop) - 1) & 0xFFFFFFFF)
    return u.view(np.float32)


def _make_in_maps(inputs):
    x = np.asarray(inputs["x"], np.float32)
    wq = np.asarray(inputs["wq"], np.float32)
    bq = np.asarray(inputs["bq"], np.float32)
    wk = np.asarray(inputs["wk"], np.float32)
    bk = np.asarray(inputs["bk"], np.float32)
    wv = np.asarray(inputs["wv"], np.float32)
    bv = np.asarray(inputs["bv"], np.float32)
    wo = np.asarray(inputs["wo"], np.float32)
    bo = np.asarray(inputs["bo"], np.float32)
    gamma = float(np.asarray(inputs["gamma"], np.float32)[0])

    s = 1.0 / math.sqrt(C)
    wpack = np.ascontiguousarray(np.hstack([
        (wq * s).T, wk.T, wv.T, (wo * gamma).T,
        (bq * s).reshape(C, 1), bk.reshape(C, 1),
        (gamma * (wo @ bv + bo)).reshape(C, 1),
    ]).astype(np.float32))

    xf = np.ascontiguousarray(x.reshape(B, C, N))
    in_maps = []
    for core in range(8):
        b, half = core // 2, core % 2
        in_maps.append({
            "x": xf[b],
            "xq": np.ascontiguousarray(xf[b][:, half * HALF:(half + 1) * HALF]),
            "wpack": wpack,
        })
    return in_maps


def _gather(results):
    out = np.empty((B, C, N), np.float32)
    for core in range(8):
        b, half = core // 2, core % 2
        out[b][:, half * HALF:(half + 1) * HALF] = results[core]["y"]
    return out.reshape(B, C, W, H)


def run(inputs, trace=False):
    """Run on the 8 NeuronCores; returns (output, exec_time_ns_or_None)."""
    from concourse.bass_utils import run_bass_kernel_spmd

    nc = _build()
    in_maps = _make_in_maps(inputs)
    res = run_bass_kernel_spmd(nc, in_maps, core_ids=list(range(8)), trace=trace)
    return _gather(res.results), res.exec_time_ns


def kernel(**inputs):
    out, _ = run(inputs)
    return out
